# revision 48
# baseline (speedup 1.0000x reference)
"""Trainium2 Bass kernel for AttributionCentroidTracker.

Reference computation (B=512, V=32768, C=16):
    Wg[b, v]   = W_eff[b, v, labels[b]]
    attr[b, v] = |sparse_vector[b, v] * Wg[b, v]|
    sums[c, v] = segment_sum(attr, labels)       # [C, V]
    mean       = sums / max(counts, 1)
    out[c]     = centroids[c]                     if counts[c] == 0
               = mean[c]                          if not initialized[c]
               = M*centroids[c] + (1-M)*mean[c]   otherwise

Device strategy (8 cores, sharded along V — per-class sums are complete
locally per V-slice, so no cross-core reduction is needed):
  - b (512) lives on the 128 SBUF partitions in 4 groups of 128.
  - W streams in as bf16 (SWDGE cast-DMA) [128, 4*VC*16] tiles.
  - |W| in place on VectorE via a sign-bit-clearing bitwise AND on an
    int32 view (2 bf16 per element, 2x mode) — ISA has no abs ALU op.
  - VectorE multiplies by |sv| broadcast along c (stride-0, 1x mode).
  - Segment-sum on TensorE with PLAIN one-hot lhsT [128,16] per batch
    group and CONTIGUOUS rhs [128,512] (v32*c16 natural layout), N=512
    matmuls accumulating over the 4 groups into 4 PSUM banks per tile.
    psum[c', (v,c)] holds per-class sums of ALL 16 channels; only the
    diagonal c'==c is wanted.
  - ScalarE evacuates each bank scaled by a_c into a bf16 staging tile;
    every TBATCH tiles the diagonal is pulled out with 16 accumulating
    selection matmuls E_cc^T @ stage[:, :, :, c] (stride-16 rhs) into a
    psum tile which VectorE adds into the f32 accumulator out_sb,
    pre-loaded with b_c*centroids (host-computed).
  - a/b host math: a = (init ? (1-M)/n : 1/n) if present else 0,
    b = (init ? M : 0) if present else 1.
"""

import os
import sys

import numpy as np

if "/opt/trn_rl_repo" not in sys.path:
    sys.path.insert(0, "/opt/trn_rl_repo")

B, V, C = 512, 32768, 16
NCORES = 8
VSH = V // NCORES            # 4096 columns of V per core
P = 128                      # SBUF partitions
BG = B // P                  # 4 batch groups
VC = 128                     # v-chunk per tile
NVC = VSH // VC              # 32 tiles per core
NSUB = 4                     # psum banks per tile (32 v each)
VSUB = VC // NSUB            # 32
TBATCH = 4                   # tiles per extraction batch
NQ = 4                       # sv quarter-loads
VQ = VSH // NQ               # 1024
STEPS_PER_EPOCH = 1000
MOMENTUM = 1.0 - 2.0 / (STEPS_PER_EPOCH + 1)

_CACHE = {}

last_exec_time_ns = None
last_results = None


def _build_nc():
    import concourse.bacc as bacc
    import concourse.tile as tile
    from concourse import mybir

    f32 = mybir.dt.float32
    bf16 = mybir.dt.bfloat16
    Copy = mybir.ActivationFunctionType.Copy
    nc = bacc.Bacc("TRN2", target_bir_lowering=False, debug=False)

    w = nc.dram_tensor("w", [B, VSH, C], f32, kind="ExternalInput")
    sv = nc.dram_tensor("sv", [B, VSH], f32, kind="ExternalInput")
    oh = nc.dram_tensor("oh", [P, BG * C], bf16, kind="ExternalInput")
    sel = nc.dram_tensor("sel", [C, C * C], bf16, kind="ExternalInput")
    centb = nc.dram_tensor("centb", [C, VSH], f32, kind="ExternalInput")
    avec = nc.dram_tensor("avec", [C, 1], f32, kind="ExternalInput")
    out = nc.dram_tensor("out", [C, VSH], f32, kind="ExternalOutput")

    # b = g*128 + p  ->  partition p, group g
    w_r = w.ap().rearrange("(g p) v c -> p g v c", p=P)      # [128, 4, VSH, 16]
    sv_r = sv.ap().rearrange("(g p) v -> p g v", p=P)        # [128, 4, VSH]

    with tile.TileContext(nc) as tc:
        with (
            tc.tile_pool(name="const", bufs=1) as cpool,
            tc.tile_pool(name="wp", bufs=7) as wpool,
            tc.tile_pool(name="stg", bufs=1) as spool,
            tc.tile_pool(name="psum", bufs=8, space="PSUM") as ppool,
        ):
            # |sv| as bf16, loaded in NQ v-quarters (quarter-major layout so
            # each quarter lands contiguously per partition) so the first
            # tile's multiply doesn't wait on the whole 8 MB transfer.
            svt = cpool.tile([P, BG * VSH], dtype=bf16)
            svt4 = svt[:].rearrange("p (q g v) -> p q g v", q=NQ, g=BG)

            def issue_sv_quarter(q):
                qsl = slice(q * VQ, (q + 1) * VQ)
                nc.gpsimd.dma_start(out=svt4[:, q], in_=sv_r[:, :, qsl])
                # abs via sign-bit clear on an int32 view (2 bf16 per elem)
                qv = svt4[:, q].bitcast(mybir.dt.int32)
                nc.vector.tensor_scalar(
                    out=qv,
                    in0=qv,
                    scalar1=0x7FFF7FFF,
                    scalar2=None,
                    op0=mybir.AluOpType.bitwise_and,
                )

            issue_sv_quarter(0)

            oh_sb = cpool.tile([P, BG * C], dtype=bf16)
            nc.sync.dma_start(out=oh_sb[:], in_=oh.ap())
            sel_sb = cpool.tile([C, C * C], dtype=bf16)
            nc.sync.dma_start(out=sel_sb[:], in_=sel.ap())
            avec_sb = cpool.tile([C, 1], dtype=f32)
            nc.sync.dma_start(out=avec_sb[:], in_=avec.ap())

            # accumulator pre-loaded with b_c * centroids
            out_sb = cpool.tile([C, VSH], dtype=f32)
            nc.sync.dma_start(out=out_sb[:], in_=centb.ap())

            def issue_w_dma(i):
                vlo = i * VC
                wt = wpool.tile([P, BG * VC * C], dtype=bf16, tag="wt")
                wt4 = wt[:].rearrange("p (g v c) -> p g v c", g=BG, v=VC)
                nc.gpsimd.dma_start(out=wt4, in_=w_r[:, :, vlo : vlo + VC, :])
                return wt, wt4

            PREFETCH = 6
            prefetched = {}
            for i in range(min(PREFETCH, NVC)):
                prefetched[i] = issue_w_dma(i)

            nsv = 1
            stage = None
            for i in range(NVC):
                vlo = i * VC
                ib = i % TBATCH

                if i + PREFETCH < NVC:
                    prefetched[i + PREFETCH] = issue_w_dma(i + PREFETCH)
                # keep sv quarters two tiles ahead of first use
                while nsv < NQ and i >= (nsv * NVC // NQ) - 2:
                    issue_sv_quarter(nsv)
                    nsv += 1

                wt, wt4 = prefetched.pop(i)

                # |W| in place: sign-bit clear on an int32 view (DVE 2x)
                wv = wt[:].bitcast(mybir.dt.int32)
                nc.vector.tensor_scalar(
                    out=wv,
                    in0=wv,
                    scalar1=0x7FFF7FFF,
                    scalar2=None,
                    op0=mybir.AluOpType.bitwise_and,
                )
                # Y = |W| * |sv|  (|sv| broadcast along c, DVE 1x)
                qi, vq = divmod(vlo, VQ)
                in1 = (
                    svt4[:, qi, :, vq : vq + VC]
                    .unsqueeze(3)
                    .broadcast_to([P, BG, VC, C])
                )
                nc.vector.tensor_tensor(
                    out=wt4, in0=wt4, in1=in1, op=mybir.AluOpType.mult
                )

                if ib == 0:
                    stage = spool.tile(
                        [C, TBATCH * VC * C], dtype=bf16, tag="stage"
                    )
                # stage layout is (c, k, v) so the diagonal matmuls read
                # contiguous [16, 512] rhs slices per class
                nchunk = TBATCH * NSUB
                stg_ev = stage[:].rearrange(
                    "q (c k v) -> q k v c", c=C, k=nchunk, v=VSUB
                )
                # segment-sum: ps[c', (v32, c)] += oh_g^T @ Y_g
                # (s outer / g inner so bank s finishes early and its
                # evacuation overlaps the remaining banks' matmuls)
                for s in range(NSUB):
                    ps = ppool.tile(
                        [C, VSUB * C],
                        dtype=mybir.dt.float32,
                        tag="ps",
                        name=f"ps{s}_{i}",
                    )
                    for g in range(BG):
                        off = g * (VC * C) + s * (VSUB * C)
                        nc.tensor.matmul(
                            out=ps[:],
                            lhsT=oh_sb[:, g * C : (g + 1) * C],
                            rhs=wt[:, off : off + VSUB * C],
                            start=(g == 0),
                            stop=(g == BG - 1),
                        )
                    # evacuate scaled by a_c into the bf16 staging tile
                    # ((c, k, v) order: strided write, contiguous diag rhs)
                    nc.scalar.activation(
                        stg_ev[:, ib * NSUB + s],
                        ps[:],
                        Copy,
                        bias=0.0,
                        scale=avec_sb[:],
                    )

                # extraction batch: diagonal (c', (v,c)) c'==c via 16
                # accumulating selection matmuls E_cc^T @ stage[:, c-block]
                if ib == TBATCH - 1:
                    ps2 = ppool.tile(
                        [C, TBATCH * VC],
                        dtype=mybir.dt.float32,
                        tag="ps",
                        name=f"ps_diag_{i}",
                    )
                    for c in range(C):
                        nc.tensor.matmul(
                            out=ps2[:],
                            lhsT=sel_sb[:, c * C : (c + 1) * C],
                            rhs=stage[
                                :, c * nchunk * VSUB : (c + 1) * nchunk * VSUB
                            ],
                            start=(c == 0),
                            stop=(c == C - 1),
                        )
                    ooff = (i - (TBATCH - 1)) * VC
                    nc.vector.tensor_tensor(
                        out=out_sb[:, ooff : ooff + TBATCH * VC],
                        in0=out_sb[:, ooff : ooff + TBATCH * VC],
                        in1=ps2[:],
                        op=mybir.AluOpType.add,
                    )
                    # write the finished slice out immediately so the
                    # final store overlaps the stream instead of tailing
                    nc.sync.dma_start(
                        out=out.ap()[:, ooff : ooff + TBATCH * VC],
                        in_=out_sb[:, ooff : ooff + TBATCH * VC],
                    )

    nc.finalize()
    return nc


def _get_nc():
    if "nc" not in _CACHE:
        _CACHE["nc"] = _build_nc()
    return _CACHE["nc"]


def kernel(sparse_vector, W_eff, labels, centroids, initialized):
    global last_exec_time_ns, last_results
    import ml_dtypes
    from concourse.bass_utils import run_bass_kernel_spmd

    sv = np.ascontiguousarray(np.asarray(sparse_vector, dtype=np.float32))
    w = np.asarray(W_eff, dtype=np.float32)
    lab = np.asarray(labels).astype(np.int64)
    cent = np.asarray(centroids, dtype=np.float32)
    init = np.asarray(initialized).astype(bool)

    # Host-side label-derived constants (tiny) — keep the program generic.
    ohm = lab[:, None] == np.arange(C)[None, :]          # [B, C] bool
    counts = ohm.sum(axis=0).astype(np.float64)          # [C]
    present = counts > 0
    safe = np.maximum(counts, 1.0)
    a = np.where(present, np.where(init, (1.0 - MOMENTUM) / safe, 1.0 / safe), 0.0)
    b = np.where(present, np.where(init, MOMENTUM, 0.0), 1.0)
    avec = a.astype(np.float32).reshape(C, 1)
    centb = (b[:, None] * cent.astype(np.float64)).astype(np.float32)  # [C, V]

    # Plain one-hot lhsT blocks: oh[p, g*C + c] = 1 iff labels[g*128+p]==c
    lab2 = lab.reshape(BG, P)                            # [g, p]
    oh = np.zeros((P, BG * C), np.float32)
    for g in range(BG):
        oh[np.arange(P), g * C + lab2[g]] = 1.0
    oh = oh.astype(ml_dtypes.bfloat16)

    # Diagonal-selection lhsT blocks: sel[p, c*C+m] = 1 iff p==c==m
    selm = np.zeros((C, C * C), np.float32)
    for c in range(C):
        selm[c, c * C + c] = 1.0
    selm = selm.astype(ml_dtypes.bfloat16)

    nc = _get_nc()
    in_maps = []
    for i in range(NCORES):
        s = i * VSH
        in_maps.append(
            {
                "w": np.ascontiguousarray(w[:, s : s + VSH, :]),
                "sv": np.ascontiguousarray(sv[:, s : s + VSH]),
                "oh": oh,
                "sel": selm,
                "centb": np.ascontiguousarray(centb[:, s : s + VSH]),
                "avec": avec,
            }
        )

    res = run_bass_kernel_spmd(nc, in_maps, core_ids=list(range(NCORES)))
    last_exec_time_ns = res.exec_time_ns
    last_results = res
    return np.concatenate([res.results[i]["out"] for i in range(NCORES)], axis=1)
